# revision 17
# baseline (speedup 1.0000x reference)
"""DifferentiableHPWL on 8 trn2 NeuronCores.

Strategy (sharded by nets, hint-compliant):
  - Host: bucket nets by pin-count, shard nets across 8 cores, compose
    slot->macro = pin_to_macro[net_to_pin] (int16 — macro ids < 32768),
    lay out per-core slot tables, the per-slot pin-offset stream
    (net-grouped layout of the replicated pin_offsets input) and the
    per-macro record table T2[v] = (x[8b], y[8b], c[8b], s[8b]) in bf16
    (64B payload rows on a 256B stride so dma_gather can address them;
    c/s are the one-hot contractions cos/sin of the 90-degree rotations).
  - Device (per core): per chunk of nets, ONE dma_gather instruction
    fetches all ~16K macro records for the chunk (vs. one indirect DMA
    per 128 slots — the SWDGE ~1us/instruction overhead was the original
    3.5ms bottleneck), spread over the 4 SWDGE queues. DVE computes
    rotated pin positions px,py for all 8 batches in bf16 ([net, pin,
    batch] layout, batch innermost so every elementwise op is contiguous),
    per-net max/min (strided-view reduces), exp via ACT (in-place), and
    per-net partial sums into persistent arenas. The cheap per-net tail
    (ln, lse combine, weighting, reduction to [128, 8] f32 partials) runs
    once over the arenas at the end.
  - Host: sum partials over partitions and cores -> (8,) float32.
"""

import numpy as np

import concourse.bass as bass
import concourse.mybir as mybir
from concourse import ap_utils
from concourse.bass_primitives import MemorySpace
from concourse.tile import TileContext
from concourse import bass_utils, library_config

F32 = mybir.dt.float32
BF16 = mybir.dt.bfloat16
I16 = mybir.dt.int16
AX = mybir.AxisListType
ALU = mybir.AluOpType
ACT = mybir.ActivationFunctionType

GAMMA = 10.0
N_CORES = 8
P = 128  # partitions
REC = 32  # record fields (bf16): x[8], y[8], c[8], s[8]
FAT = 128  # fat row stride in bf16 elements (256B, dma_gather requirement)


def _patch_tile_drain():
    """This walrus lowers InstDrain to a TPB_CTRL form with too few sync-wait
    slots; hoist the final drain's waits onto single-wait nops instead."""
    from concourse.vector_clock import ScopedClock

    if getattr(TileContext, "_drain_patched", False):
        return

    def _drain_and_barrier(self, tick_clock, wait_clock):
        nc = self.nc
        carrier = nc.sync.nop(nofuse=True, hint="drain_wait_carrier")
        wait_clock.add_sem_waits(
            carrier.ins, ScopedClock({None: tick_clock.global_clock})
        )
        waits = list(carrier.ins.sync_info.on_wait) if carrier.ins.sync_info else []
        if len(waits) > 1:
            carrier.ins.sync_info = mybir.SyncInfo(on_wait=[waits[0]], on_update=[])
            for w in waits[1:]:
                n2 = nc.sync.nop(nofuse=True, hint="drain_wait_extra")
                n2.ins.sync_info = mybir.SyncInfo(on_wait=[w], on_update=[])
        nc.sync.drain()
        nc.all_engine_barrier()
        popped = nc._tile_sem_poison_stack.pop()
        assert popped is self._sem_poison
        nc.clear_and_free_semaphores(list(self.sems.allocated().values()))
        nc.all_engine_barrier()

    TileContext._drain_and_barrier = _drain_and_barrier
    TileContext._drain_patched = True


def _split_excess_waits(nc, dma_limit=1, other_limit=1):
    """walrus here rejects DMA instructions with >1 sync wait (and drains with
    >1). Hoist excess waits onto same-engine NoOp carriers inserted before the
    instruction — the sequencer executes carrier waits first, preserving
    semantics."""
    ctr = 0
    dma_types = (mybir.InstDMACopy, mybir.InstDrain, mybir.InstDMAGatherAnt)
    for f in nc.m.functions:
        for bb in f.blocks:
            out = []
            changed = False
            for inst in bb.instructions:
                si = inst.sync_info
                waits = list(si.on_wait) if si and si.on_wait else []
                limit = dma_limit if isinstance(inst, dma_types) else other_limit
                if len(waits) > limit:
                    keep = waits[len(waits) - limit:]
                    for w in waits[: len(waits) - limit]:
                        nop = mybir.InstNoOp(name=f"waitsplit-{ctr}")
                        ctr += 1
                        nop.engine = inst.engine
                        nop.sync_info = mybir.SyncInfo(on_wait=[w], on_update=[])
                        nc.register_instruction(nop, overwrite=True)
                        out.append(nop)
                    inst.sync_info = mybir.SyncInfo(
                        on_wait=keep,
                        on_update=list(si.on_update) if si.on_update else [],
                    )
                    changed = True
                out.append(inst)
            if changed:
                bb.instructions = out
    return ctr


def _dma_gather(g, out_ap, in_ap, idxs_ap, num_idxs, elem_size, elem_step,
                queue_num=0, reg_cache=None):
    """nc.gpsimd.dma_gather without the elem%256 assert — the firmware's
    non-transpose path only requires the table row *stride* to be a 256B
    multiple (stride_bytes_256 descriptor field); the transferred elem size
    is free. single_packet=False (concat-all overflows the 64-desc packet
    limit and wedges the SDMA)."""
    nc = g.bass
    assert idxs_ap.dtype == I16
    assert in_ap.space == MemorySpace.DRAM
    assert out_ap.space == MemorySpace.SBUF
    assert idxs_ap.space == MemorySpace.SBUF
    assert ap_utils.ap_is_contiguous(out_ap.ap[1:])
    assert ap_utils.ap_is_contiguous(idxs_ap.ap[1:])
    assert in_ap.ap[-1][1] == elem_size
    assert out_ap.ap[-1][1] == elem_size
    assert out_ap.ap[0][1] * out_ap.ap[1][1] == ((num_idxs + 127) // 128) * 128
    assert in_ap.ap[0][0] == elem_step
    stride_bytes = elem_step * mybir.dt.size(in_ap.dtype)
    assert stride_bytes % 256 == 0 and stride_bytes // 256 < 256
    _in_ap = g.lower_ap_dma(in_ap, for_custom_bir_dma=True)
    _idxs_ap = g.lower_ap(idxs_ap)
    _out_ap = g.lower_ap(out_ap)
    if reg_cache is not None and num_idxs in reg_cache:
        reg = reg_cache[num_idxs]
    else:
        reg = g.to_reg(num_idxs)
        if reg_cache is not None:
            reg_cache[num_idxs] = reg
    return g.add_instruction(
        mybir.InstDMAGatherAnt(
            name=nc.get_next_instruction_name(),
            ins=[*_in_ap, _idxs_ap, g.lower_val_access(reg)],
            outs=[_out_ap],
            transpose=False,
            num_idxs=num_idxs,
            elem_size=elem_size,
            stride_bytes_256=stride_bytes // 256,
            gen_mode=0,
            single_packet=False,
            queue_num=queue_num,
            sbuf_tokens_per_rank=0,
            sbuf_free_dim_per_rank=0,
            sbuf_free_dim_pad_per_rank=0,
            sbuf_byte_offset=0,
        )
    )


def build_program(vpad, ppad, chunk_plan, tot_slot, tot_g, rep=1):
    """Build the SPMD Bass program.

    vpad: padded macro count (multiple of 128); ppad: unused (layout compat).
    chunk_plan: list of (k, g, slot_off, g_off) chunks; cs = g*k <= 127.
    tot_slot: total slots per partition; tot_g: total net-groups/partition.
    rep: repeat the whole compute (timing builds only).
    """
    _patch_tile_drain()
    nc = bass.Bass("TRN2", target_bir_lowering=False, debug=False,
                   num_swdge_queues=4)

    t1e = nc.dram_tensor("t1e", [P, tot_slot * 2], BF16, kind="ExternalInput")
    t2f = nc.dram_tensor("t2f", [vpad, FAT], BF16, kind="ExternalInput")
    idx16 = nc.dram_tensor("idx16", [P, tot_slot * 8], I16, kind="ExternalInput")
    w_all = nc.dram_tensor("w_all", [P, tot_g], BF16, kind="ExternalInput")
    out = nc.dram_tensor("acc", [P, 8], F32, kind="ExternalOutput")

    G8 = tot_g * 8

    with TileContext(nc) as tc:
        with (
            tc.tile_pool(name="persist", bufs=1) as pp,
            tc.tile_pool(name="work", bufs=4) as wp,
        ):
            nc.gpsimd.load_library(library_config.mlp)
            # ---- persistent loads ----
            idx_t = pp.tile([P, tot_slot * 8], I16)
            nc.sync.dma_start(idx_t[:], idx16.ap())
            t1_t = pp.tile([P, tot_slot * 2], BF16)
            nc.sync.dma_start(t1_t[:], t1e.ap())
            w_t = pp.tile([P, tot_g], BF16)
            nc.sync.dma_start(w_t[:], w_all.ap())
            acc = pp.tile([P, 8], F32)
            # per-net arenas: [coord][net-group, batch] for Mx/mn/Sx/Sn; the
            # cheap per-net tail math runs once over these per rep instead of
            # as ~10 tiny engine ops per chunk.
            aM = pp.tile([P, 2 * G8], BF16)
            am = pp.tile([P, 2 * G8], BF16)
            aSx = pp.tile([P, 2 * G8], BF16)
            aSn = pp.tile([P, 2 * G8], BF16)

            reg_cache = {}
            # Execute chunks largest-first (short pipeline drain) with
            # greedy LPT queue assignment — chunk sizes vary ~5x, so ci%4
            # round-robin left one SWDGE queue with ~1.7x the descriptors
            # of another.
            order = sorted(range(len(chunk_plan)),
                           key=lambda i: -chunk_plan[i][0] * chunk_plan[i][1])
            qload = [0] * 4
            qassign = {}
            for i in order:
                q = min(range(4), key=lambda x: qload[x])
                qassign[i] = q
                qload[q] += chunk_plan[i][0] * chunk_plan[i][1]
            for _ in range(rep):
                # ---- chunk loop ----
                for ci in order:
                    (k, g, slot_off, g_off) = chunk_plan[ci]
                    cs = g * k  # slots per partition this chunk
                    n_idx = cs * P
                    rec = wp.tile([P, cs * REC], BF16, tag="rec")
                    _dma_gather(
                        nc.gpsimd,
                        out_ap=rec.rearrange("p (c e) -> p c e", e=REC),
                        in_ap=t2f.ap()[:, 0:REC],
                        idxs_ap=idx_t[:, 8 * slot_off: 8 * (slot_off + cs)],
                        num_idxs=n_idx, elem_size=REC, elem_step=FAT,
                        queue_num=qassign[ci], reg_cache=reg_cache,
                    )

                    r5 = rec.rearrange("p (g j c) -> p g j c", g=g, j=k)
                    Xv = r5[:, :, :, 0:8]
                    Yv = r5[:, :, :, 8:16]
                    Cv = r5[:, :, :, 16:24]
                    Sv = r5[:, :, :, 24:32]
                    r1f = t1_t[:, 2 * slot_off: 2 * (slot_off + cs)].rearrange(
                        "p (g j f) -> p g j f", g=g, j=k)
                    oxv = r1f[:, :, :, 0:1].to_broadcast([P, g, k, 8])
                    oyv = r1f[:, :, :, 1:2].to_broadcast([P, g, k, 8])

                    # slot tiles are [g, j, b], b innermost: every build op
                    # contiguous; per-net reduces use a [g, b, j] strided view.
                    pv = wp.tile([P, 2 * cs * 8], BF16, tag="pv")
                    ta = wp.tile([P, cs * 8], BF16, tag="ta")
                    tar = ta.rearrange("p (g j b) -> p g j b", g=g, j=k)
                    tb = wp.tile([P, cs * 8], BF16, tag="tb")
                    tbr = tb.rearrange("p (g j b) -> p g j b", g=g, j=k)
                    pvc = [
                        pv[:, c * cs * 8:(c + 1) * cs * 8].rearrange(
                            "p (g j b) -> p g j b", g=g, j=k)
                        for c in range(2)
                    ]

                    nc.vector.tensor_tensor(out=tar, in0=Cv, in1=oxv,
                                            op=ALU.mult)
                    nc.vector.tensor_tensor(out=tbr, in0=Sv, in1=oyv,
                                            op=ALU.mult)
                    nc.vector.tensor_tensor(out=tar, in0=tar, in1=Xv,
                                            op=ALU.add)
                    nc.vector.tensor_tensor(out=pvc[0], in0=tar, in1=tbr,
                                            op=ALU.subtract)
                    nc.vector.tensor_tensor(out=tar, in0=Sv, in1=oxv,
                                            op=ALU.mult)
                    nc.vector.tensor_tensor(out=tbr, in0=Cv, in1=oyv,
                                            op=ALU.mult)
                    nc.vector.tensor_tensor(out=tar, in0=tar, in1=Yv,
                                            op=ALU.add)
                    nc.vector.tensor_tensor(out=pvc[1], in0=tar, in1=tbr,
                                            op=ALU.add)

                    for c in range(2):
                        off = c * G8 + g_off * 8
                        Mx = aM[:, off:off + g * 8]
                        mn = am[:, off:off + g * 8]
                        src_t = pvc[c].transpose([0, 1, 3, 2])  # [P, g, b, j]
                        nc.vector.tensor_reduce(out=Mx, in_=src_t, axis=AX.X,
                                                op=ALU.max)
                        nc.vector.tensor_reduce(out=mn, in_=src_t, axis=AX.X,
                                                op=ALU.min)
                        Mb = Mx.rearrange("p (g b) -> p g b", g=g).unsqueeze(
                            2).to_broadcast([P, g, k, 8])
                        mb = mn.rearrange("p (g b) -> p g b", g=g).unsqueeze(
                            2).to_broadcast([P, g, k, 8])
                        d1 = wp.tile([P, cs * 8], BF16, tag=f"d1{c}")
                        d1r = d1.rearrange("p (g j b) -> p g j b", g=g, j=k)
                        d1t = d1r.transpose([0, 1, 3, 2])
                        d2 = wp.tile([P, cs * 8], BF16, tag=f"d2{c}")
                        d2r = d2.rearrange("p (g j b) -> p g j b", g=g, j=k)
                        d2t = d2r.transpose([0, 1, 3, 2])
                        nc.vector.tensor_tensor(out=d1r, in0=pvc[c], in1=Mb,
                                                op=ALU.subtract)
                        nc.scalar.activation(out=d1[:], in_=d1[:],
                                             func=ACT.Exp, scale=GAMMA)
                        nc.vector.tensor_tensor(out=d2r, in0=pvc[c], in1=mb,
                                                op=ALU.subtract)
                        nc.scalar.activation(out=d2[:], in_=d2[:],
                                             func=ACT.Exp, scale=-GAMMA)
                        with nc.allow_low_precision(reason="<=16-term sums"):
                            nc.vector.tensor_reduce(
                                out=aSx[:, off:off + g * 8], in_=d1t,
                                axis=AX.X, op=ALU.add)
                            nc.vector.tensor_reduce(
                                out=aSn[:, off:off + g * 8], in_=d2t,
                                axis=AX.X, op=ALU.add)

                # ---- per-net tail, once over the whole arenas ----
                nc.scalar.activation(out=aSx[:], in_=aSx[:], func=ACT.Ln)
                nc.scalar.activation(out=aSn[:], in_=aSn[:], func=ACT.Ln)
                nc.vector.tensor_tensor(out=aSx[:], in0=aSx[:], in1=aSn[:],
                                        op=ALU.add)
                nc.vector.tensor_tensor(out=aM[:], in0=aM[:], in1=am[:],
                                        op=ALU.subtract)
                nc.vector.scalar_tensor_tensor(
                    out=aM[:], in0=aSx[:], scalar=1.0 / GAMMA, in1=aM[:],
                    op0=ALU.mult, op1=ALU.add,
                )
                nc.vector.tensor_tensor(out=aM[:, 0:G8], in0=aM[:, 0:G8],
                                        in1=aM[:, G8:2 * G8], op=ALU.add)
                wbig = w_t.unsqueeze(2).to_broadcast([P, tot_g, 8])
                wl = aM[:, 0:G8].rearrange("p (g b) -> p g b", g=tot_g)
                nc.vector.tensor_tensor(out=wl, in0=wl, in1=wbig, op=ALU.mult)
                nc.vector.tensor_reduce(
                    out=acc[:], in_=wl.transpose([0, 2, 1]), axis=AX.X,
                    op=ALU.add,
                )

            nc.sync.dma_start(out.ap(), acc[:])
    _split_excess_waits(nc)
    from concourse.library_overlay import lower_extended_insts
    lower_extended_insts(nc)
    return nc


def prep_host(positions, pin_offsets, rotation_onehot, net_weights,
              net_to_pin, pin_to_macro):
    """Host-side sharding/layout. Returns (meta, in_maps)."""
    B, V, _ = positions.shape
    Pn = pin_offsets.shape[0]
    N, M = net_to_pin.shape
    bf16 = bfloat16_np()

    vpad = ((V + 1 + P - 1) // P) * P  # +1 pad macro row
    ppad = Pn + 1                      # +1 pad pin row
    pad_mac = V
    pad_pin = Pn

    n2p = net_to_pin.astype(np.int32)
    p2m = np.concatenate(
        [pin_to_macro.astype(np.int32), np.array([pad_mac], np.int32)]
    )
    t1pad = np.concatenate(
        [pin_offsets.astype(np.float32), np.zeros((1, 2), np.float32)]
    )

    # replicated fat macro-record table (bf16):
    # x[8], y[8], c = oh0-oh2, s = oh1-oh3 per batch
    t2f = np.zeros((vpad, FAT), bf16)
    t2f[:V, 0:8] = positions[:, :, 0].T.astype(bf16)
    t2f[:V, 8:16] = positions[:, :, 1].T.astype(bf16)
    ohT = rotation_onehot.transpose(1, 0, 2)  # (V, B, 4)
    t2f[:V, 16:24] = (ohT[:, :, 0] - ohT[:, :, 2]).astype(bf16)
    t2f[:V, 24:32] = (ohT[:, :, 1] - ohT[:, :, 3]).astype(bf16)

    lengths = (n2p >= 0).sum(axis=1)

    # shard nets contiguously
    per = (N + N_CORES - 1) // N_CORES
    shards = [(c * per, min((c + 1) * per, N)) for c in range(N_CORES)]

    # bucket counts per core -> global G_k
    ks = range(1, M + 1)
    counts = np.zeros((N_CORES, M + 1), np.int64)
    for c, (a, b) in enumerate(shards):
        cnt = np.bincount(lengths[a:b], minlength=M + 1)
        counts[c] = cnt
    gk = {k: int(-(-counts[:, k].max() // P)) for k in ks if counts[:, k].max() > 0}

    # chunk plan: cs = g*k <= 127 so each chunk is one dma_gather
    # (<= 16256 indices; the SWDGE descriptor carveout fits ~16336).
    chunk_plan = []
    slot_off = 0
    g_off = 0
    bucket_offs = {}
    for k in sorted(gk):
        g_total = gk[k]
        gmax = max(1, 127 // k)
        bucket_offs[k] = (slot_off, g_off)
        g_done = 0
        while g_done < g_total:
            g = min(gmax, g_total - g_done)
            chunk_plan.append((k, g, slot_off, g_off))
            slot_off += g * k
            g_off += g
            g_done += g
    tot_slot = slot_off
    tot_g = g_off

    # per-core slot tables
    in_maps = []
    for c, (a, b) in enumerate(shards):
        mac_all = np.full((P, tot_slot), pad_mac, np.int32)
        w_core = np.zeros((P, tot_g), np.float32)
        pin_all = np.full((P, tot_slot), pad_pin, np.int32)
        ln = lengths[a:b]
        for k in sorted(gk):
            so, go = bucket_offs[k]
            sel = np.nonzero(ln == k)[0]
            nk = len(sel)
            if nk == 0:
                continue
            gkk = gk[k]
            ids = n2p[a:b][sel][:, :k]               # (nk, k) valid prefix
            w = net_weights[a:b][sel].astype(np.float32)
            idsp = np.full((gkk * P, k), pad_pin, np.int32)
            idsp[:nk] = ids
            wp_ = np.zeros((gkk * P,), np.float32)
            wp_[:nk] = w
            # net r -> (g=r//P, p=r%P)
            pin_all[:, so:so + gkk * k] = (
                idsp.reshape(gkk, P, k).transpose(1, 0, 2).reshape(P, gkk * k)
            )
            mac_all[:, so:so + gkk * k] = p2m[pin_all[:, so:so + gkk * k]]
            w_core[:, go:go + gkk] = wp_.reshape(gkk, P).T

        # int16 wrapped index stream for dma_gather: list position
        # i = col*128 + p; wrapped tile [128, 8*tot_slot] with
        # tile[q, s] = idx_list[s*16 + q%16].
        idx_list = mac_all.T.ravel().astype(np.int16)         # [tot_slot*128]
        idx16 = np.tile(idx_list.reshape(tot_slot * 8, 16).T, (8, 1))
        # per-slot pin offsets, net-grouped dense stream (bf16)
        t1e = t1pad[pin_all].astype(np.float32).reshape(P, tot_slot * 2)

        in_maps.append({
            "t1e": t1e.astype(bf16), "t2f": t2f,
            "idx16": idx16.astype(np.int16),
            "w_all": w_core.astype(bf16),
        })

    meta = (vpad, ppad, tuple(chunk_plan), tot_slot, tot_g)
    return meta, in_maps


def bfloat16_np():
    import ml_dtypes
    return ml_dtypes.bfloat16


_prog_cache = {}


def kernel(**inputs):
    meta, in_maps = prep_host(
        np.asarray(inputs["positions"]),
        np.asarray(inputs["pin_offsets"]),
        np.asarray(inputs["rotation_onehot"]),
        np.asarray(inputs["net_weights"]),
        np.asarray(inputs["net_to_pin"]),
        np.asarray(inputs["pin_to_macro"]),
    )
    if meta not in _prog_cache:
        _prog_cache[meta] = build_program(*meta)
    nc = _prog_cache[meta]
    res = bass_utils.run_bass_kernel_spmd(nc, in_maps, core_ids=list(range(N_CORES)))
    total = np.zeros(8, np.float64)
    for r in res.results:
        total += r["acc"].astype(np.float64).sum(axis=0)
    return total.astype(np.float32)


# revision 18
# speedup vs baseline: 1.0401x; 1.0401x over previous
"""DifferentiableHPWL on 8 trn2 NeuronCores.

Strategy (sharded by nets, hint-compliant):
  - Host: bucket nets by pin-count, shard nets across 8 cores, compose
    slot->macro = pin_to_macro[net_to_pin] (int16 — macro ids < 32768),
    lay out per-core slot tables, the per-slot pin-offset stream
    (net-grouped layout of the replicated pin_offsets input) and the
    per-macro record table T2[v] = (x[8b], y[8b], c[8b], s[8b]) in bf16
    (64B payload rows on a 256B stride so dma_gather can address them;
    c/s are the one-hot contractions cos/sin of the 90-degree rotations).
  - Device (per core): per chunk of nets, ONE dma_gather instruction
    fetches all ~16K macro records for the chunk (vs. one indirect DMA
    per 128 slots — the SWDGE ~1us/instruction overhead was the original
    3.5ms bottleneck), spread over the 4 SWDGE queues. DVE computes
    rotated pin positions px,py for all 8 batches in bf16 ([net, pin,
    batch] layout, batch innermost so every elementwise op is contiguous),
    per-net max/min (strided-view reduces), exp via ACT (in-place), and
    per-net partial sums into persistent arenas. The cheap per-net tail
    (ln, lse combine, weighting, reduction to [128, 8] f32 partials) runs
    once over the arenas at the end.
  - Host: sum partials over partitions and cores -> (8,) float32.
"""

import numpy as np

import concourse.bass as bass
import concourse.mybir as mybir
from concourse import ap_utils
from concourse.bass_primitives import MemorySpace
from concourse.tile import TileContext
from concourse import bass_utils, library_config

F32 = mybir.dt.float32
BF16 = mybir.dt.bfloat16
I16 = mybir.dt.int16
AX = mybir.AxisListType
ALU = mybir.AluOpType
ACT = mybir.ActivationFunctionType

GAMMA = 10.0
N_CORES = 8
P = 128  # partitions
REC = 32  # record fields (bf16): x[8], y[8], c[8], s[8]
FAT = 128  # fat row stride in bf16 elements (256B, dma_gather requirement)


def _patch_tile_drain():
    """This walrus lowers InstDrain to a TPB_CTRL form with too few sync-wait
    slots; hoist the final drain's waits onto single-wait nops instead."""
    from concourse.vector_clock import ScopedClock

    if getattr(TileContext, "_drain_patched", False):
        return

    def _drain_and_barrier(self, tick_clock, wait_clock):
        nc = self.nc
        carrier = nc.sync.nop(nofuse=True, hint="drain_wait_carrier")
        wait_clock.add_sem_waits(
            carrier.ins, ScopedClock({None: tick_clock.global_clock})
        )
        waits = list(carrier.ins.sync_info.on_wait) if carrier.ins.sync_info else []
        if len(waits) > 1:
            carrier.ins.sync_info = mybir.SyncInfo(on_wait=[waits[0]], on_update=[])
            for w in waits[1:]:
                n2 = nc.sync.nop(nofuse=True, hint="drain_wait_extra")
                n2.ins.sync_info = mybir.SyncInfo(on_wait=[w], on_update=[])
        nc.sync.drain()
        nc.all_engine_barrier()
        popped = nc._tile_sem_poison_stack.pop()
        assert popped is self._sem_poison
        nc.clear_and_free_semaphores(list(self.sems.allocated().values()))
        nc.all_engine_barrier()

    TileContext._drain_and_barrier = _drain_and_barrier
    TileContext._drain_patched = True


def _split_excess_waits(nc, dma_limit=1, other_limit=1):
    """walrus here rejects DMA instructions with >1 sync wait (and drains with
    >1). Hoist excess waits onto same-engine NoOp carriers inserted before the
    instruction — the sequencer executes carrier waits first, preserving
    semantics."""
    ctr = 0
    dma_types = (mybir.InstDMACopy, mybir.InstDrain, mybir.InstDMAGatherAnt)
    for f in nc.m.functions:
        for bb in f.blocks:
            out = []
            changed = False
            for inst in bb.instructions:
                si = inst.sync_info
                waits = list(si.on_wait) if si and si.on_wait else []
                limit = dma_limit if isinstance(inst, dma_types) else other_limit
                if len(waits) > limit:
                    keep = waits[len(waits) - limit:]
                    for w in waits[: len(waits) - limit]:
                        nop = mybir.InstNoOp(name=f"waitsplit-{ctr}")
                        ctr += 1
                        nop.engine = inst.engine
                        nop.sync_info = mybir.SyncInfo(on_wait=[w], on_update=[])
                        nc.register_instruction(nop, overwrite=True)
                        out.append(nop)
                    inst.sync_info = mybir.SyncInfo(
                        on_wait=keep,
                        on_update=list(si.on_update) if si.on_update else [],
                    )
                    changed = True
                out.append(inst)
            if changed:
                bb.instructions = out
    return ctr


def _dma_gather(g, out_ap, in_ap, idxs_ap, num_idxs, elem_size, elem_step,
                queue_num=0, reg_cache=None):
    """nc.gpsimd.dma_gather without the elem%256 assert — the firmware's
    non-transpose path only requires the table row *stride* to be a 256B
    multiple (stride_bytes_256 descriptor field); the transferred elem size
    is free. single_packet=False (concat-all overflows the 64-desc packet
    limit and wedges the SDMA)."""
    nc = g.bass
    assert idxs_ap.dtype == I16
    assert in_ap.space == MemorySpace.DRAM
    assert out_ap.space == MemorySpace.SBUF
    assert idxs_ap.space == MemorySpace.SBUF
    assert ap_utils.ap_is_contiguous(out_ap.ap[1:])
    assert ap_utils.ap_is_contiguous(idxs_ap.ap[1:])
    assert in_ap.ap[-1][1] == elem_size
    assert out_ap.ap[-1][1] == elem_size
    assert out_ap.ap[0][1] * out_ap.ap[1][1] == ((num_idxs + 127) // 128) * 128
    assert in_ap.ap[0][0] == elem_step
    stride_bytes = elem_step * mybir.dt.size(in_ap.dtype)
    assert stride_bytes % 256 == 0 and stride_bytes // 256 < 256
    _in_ap = g.lower_ap_dma(in_ap, for_custom_bir_dma=True)
    _idxs_ap = g.lower_ap(idxs_ap)
    _out_ap = g.lower_ap(out_ap)
    if reg_cache is not None and num_idxs in reg_cache:
        reg = reg_cache[num_idxs]
    else:
        reg = g.to_reg(num_idxs)
        if reg_cache is not None:
            reg_cache[num_idxs] = reg
    return g.add_instruction(
        mybir.InstDMAGatherAnt(
            name=nc.get_next_instruction_name(),
            ins=[*_in_ap, _idxs_ap, g.lower_val_access(reg)],
            outs=[_out_ap],
            transpose=False,
            num_idxs=num_idxs,
            elem_size=elem_size,
            stride_bytes_256=stride_bytes // 256,
            gen_mode=0,
            single_packet=False,
            queue_num=queue_num,
            sbuf_tokens_per_rank=0,
            sbuf_free_dim_per_rank=0,
            sbuf_free_dim_pad_per_rank=0,
            sbuf_byte_offset=0,
        )
    )


def build_program(vpad, ppad, chunk_plan, tot_slot, tot_g, rep=1):
    """Build the SPMD Bass program.

    vpad: padded macro count (multiple of 128); ppad: unused (layout compat).
    chunk_plan: list of (k, g, slot_off, g_off) chunks; cs = g*k <= 127.
    tot_slot: total slots per partition; tot_g: total net-groups/partition.
    rep: repeat the whole compute (timing builds only).
    """
    _patch_tile_drain()
    nc = bass.Bass("TRN2", target_bir_lowering=False, debug=False,
                   num_swdge_queues=4)

    t1e = nc.dram_tensor("t1e", [P, tot_slot * 2], BF16, kind="ExternalInput")
    t2f = nc.dram_tensor("t2f", [vpad, FAT], BF16, kind="ExternalInput")
    idx16 = nc.dram_tensor("idx16", [P, tot_slot * 8], I16, kind="ExternalInput")
    w_all = nc.dram_tensor("w_all", [P, tot_g], BF16, kind="ExternalInput")
    out = nc.dram_tensor("acc", [P, 8], F32, kind="ExternalOutput")

    G8 = tot_g * 8

    with TileContext(nc) as tc:
        with (
            tc.tile_pool(name="persist", bufs=1) as pp,
            tc.tile_pool(name="work", bufs=4) as wp,
        ):
            nc.gpsimd.load_library(library_config.mlp)
            # ---- persistent loads ----
            idx_t = pp.tile([P, tot_slot * 8], I16)
            nc.sync.dma_start(idx_t[:], idx16.ap())
            t1_t = pp.tile([P, tot_slot * 2], BF16)
            nc.sync.dma_start(t1_t[:], t1e.ap())
            w_t = pp.tile([P, tot_g], BF16)
            nc.sync.dma_start(w_t[:], w_all.ap())
            acc = pp.tile([P, 8], F32)
            # per-net arenas: [coord][net-group, batch] for Mx/mn/Sx/Sn; the
            # cheap per-net tail math runs once over these per rep instead of
            # as ~10 tiny engine ops per chunk.
            aM = pp.tile([P, 2 * G8], BF16)
            am = pp.tile([P, 2 * G8], BF16)
            aSx = pp.tile([P, 2 * G8], BF16)
            aSn = pp.tile([P, 2 * G8], BF16)

            reg_cache = {}
            # Execute chunks largest-first (short pipeline drain) with
            # greedy LPT queue assignment — chunk sizes vary ~5x, so ci%4
            # round-robin left one SWDGE queue with ~1.7x the descriptors
            # of another.
            order = list(range(len(chunk_plan)))
            qassign = {i: i % 4 for i in order}
            for _ in range(rep):
                # ---- chunk loop ----
                for ci in order:
                    (k, g, slot_off, g_off) = chunk_plan[ci]
                    cs = g * k  # slots per partition this chunk
                    n_idx = cs * P
                    rec = wp.tile([P, cs * REC], BF16, tag="rec")
                    _dma_gather(
                        nc.gpsimd,
                        out_ap=rec.rearrange("p (c e) -> p c e", e=REC),
                        in_ap=t2f.ap()[:, 0:REC],
                        idxs_ap=idx_t[:, 8 * slot_off: 8 * (slot_off + cs)],
                        num_idxs=n_idx, elem_size=REC, elem_step=FAT,
                        queue_num=qassign[ci], reg_cache=reg_cache,
                    )

                    r5 = rec.rearrange("p (g j c) -> p g j c", g=g, j=k)
                    Xv = r5[:, :, :, 0:8]
                    Yv = r5[:, :, :, 8:16]
                    Cv = r5[:, :, :, 16:24]
                    Sv = r5[:, :, :, 24:32]
                    r1f = t1_t[:, 2 * slot_off: 2 * (slot_off + cs)].rearrange(
                        "p (g j f) -> p g j f", g=g, j=k)
                    oxv = r1f[:, :, :, 0:1].to_broadcast([P, g, k, 8])
                    oyv = r1f[:, :, :, 1:2].to_broadcast([P, g, k, 8])

                    # slot tiles are [g, j, b], b innermost: every build op
                    # contiguous; per-net reduces use a [g, b, j] strided view.
                    pv = wp.tile([P, 2 * cs * 8], BF16, tag="pv")
                    ta = wp.tile([P, cs * 8], BF16, tag="ta")
                    tar = ta.rearrange("p (g j b) -> p g j b", g=g, j=k)
                    tb = wp.tile([P, cs * 8], BF16, tag="tb")
                    tbr = tb.rearrange("p (g j b) -> p g j b", g=g, j=k)
                    pvc = [
                        pv[:, c * cs * 8:(c + 1) * cs * 8].rearrange(
                            "p (g j b) -> p g j b", g=g, j=k)
                        for c in range(2)
                    ]

                    nc.vector.tensor_tensor(out=tar, in0=Cv, in1=oxv,
                                            op=ALU.mult)
                    nc.vector.tensor_tensor(out=tbr, in0=Sv, in1=oyv,
                                            op=ALU.mult)
                    nc.vector.tensor_tensor(out=tar, in0=tar, in1=Xv,
                                            op=ALU.add)
                    nc.vector.tensor_tensor(out=pvc[0], in0=tar, in1=tbr,
                                            op=ALU.subtract)
                    nc.vector.tensor_tensor(out=tar, in0=Sv, in1=oxv,
                                            op=ALU.mult)
                    nc.vector.tensor_tensor(out=tbr, in0=Cv, in1=oyv,
                                            op=ALU.mult)
                    nc.vector.tensor_tensor(out=tar, in0=tar, in1=Yv,
                                            op=ALU.add)
                    nc.vector.tensor_tensor(out=pvc[1], in0=tar, in1=tbr,
                                            op=ALU.add)

                    for c in range(2):
                        off = c * G8 + g_off * 8
                        Mx = aM[:, off:off + g * 8]
                        mn = am[:, off:off + g * 8]
                        src_t = pvc[c].transpose([0, 1, 3, 2])  # [P, g, b, j]
                        nc.vector.tensor_reduce(out=Mx, in_=src_t, axis=AX.X,
                                                op=ALU.max)
                        nc.vector.tensor_reduce(out=mn, in_=src_t, axis=AX.X,
                                                op=ALU.min)
                        Mb = Mx.rearrange("p (g b) -> p g b", g=g).unsqueeze(
                            2).to_broadcast([P, g, k, 8])
                        mb = mn.rearrange("p (g b) -> p g b", g=g).unsqueeze(
                            2).to_broadcast([P, g, k, 8])
                        d1 = wp.tile([P, cs * 8], BF16, tag=f"d1{c}")
                        d1r = d1.rearrange("p (g j b) -> p g j b", g=g, j=k)
                        d1t = d1r.transpose([0, 1, 3, 2])
                        d2 = wp.tile([P, cs * 8], BF16, tag=f"d2{c}")
                        d2r = d2.rearrange("p (g j b) -> p g j b", g=g, j=k)
                        d2t = d2r.transpose([0, 1, 3, 2])
                        nc.vector.tensor_tensor(out=d1r, in0=pvc[c], in1=Mb,
                                                op=ALU.subtract)
                        nc.scalar.activation(out=d1[:], in_=d1[:],
                                             func=ACT.Exp, scale=GAMMA)
                        nc.vector.tensor_tensor(out=d2r, in0=pvc[c], in1=mb,
                                                op=ALU.subtract)
                        nc.scalar.activation(out=d2[:], in_=d2[:],
                                             func=ACT.Exp, scale=-GAMMA)
                        with nc.allow_low_precision(reason="<=16-term sums"):
                            nc.vector.tensor_reduce(
                                out=aSx[:, off:off + g * 8], in_=d1t,
                                axis=AX.X, op=ALU.add)
                            nc.vector.tensor_reduce(
                                out=aSn[:, off:off + g * 8], in_=d2t,
                                axis=AX.X, op=ALU.add)

                # ---- per-net tail, once over the whole arenas ----
                nc.scalar.activation(out=aSx[:], in_=aSx[:], func=ACT.Ln)
                nc.scalar.activation(out=aSn[:], in_=aSn[:], func=ACT.Ln)
                nc.vector.tensor_tensor(out=aSx[:], in0=aSx[:], in1=aSn[:],
                                        op=ALU.add)
                nc.vector.tensor_tensor(out=aM[:], in0=aM[:], in1=am[:],
                                        op=ALU.subtract)
                nc.vector.scalar_tensor_tensor(
                    out=aM[:], in0=aSx[:], scalar=1.0 / GAMMA, in1=aM[:],
                    op0=ALU.mult, op1=ALU.add,
                )
                nc.vector.tensor_tensor(out=aM[:, 0:G8], in0=aM[:, 0:G8],
                                        in1=aM[:, G8:2 * G8], op=ALU.add)
                wbig = w_t.unsqueeze(2).to_broadcast([P, tot_g, 8])
                wl = aM[:, 0:G8].rearrange("p (g b) -> p g b", g=tot_g)
                nc.vector.tensor_tensor(out=wl, in0=wl, in1=wbig, op=ALU.mult)
                nc.vector.tensor_reduce(
                    out=acc[:], in_=wl.transpose([0, 2, 1]), axis=AX.X,
                    op=ALU.add,
                )

            nc.sync.dma_start(out.ap(), acc[:])
    _split_excess_waits(nc)
    from concourse.library_overlay import lower_extended_insts
    lower_extended_insts(nc)
    return nc


def prep_host(positions, pin_offsets, rotation_onehot, net_weights,
              net_to_pin, pin_to_macro):
    """Host-side sharding/layout. Returns (meta, in_maps)."""
    B, V, _ = positions.shape
    Pn = pin_offsets.shape[0]
    N, M = net_to_pin.shape
    bf16 = bfloat16_np()

    vpad = ((V + 1 + P - 1) // P) * P  # +1 pad macro row
    ppad = Pn + 1                      # +1 pad pin row
    pad_mac = V
    pad_pin = Pn

    n2p = net_to_pin.astype(np.int32)
    p2m = np.concatenate(
        [pin_to_macro.astype(np.int32), np.array([pad_mac], np.int32)]
    )
    t1pad = np.concatenate(
        [pin_offsets.astype(np.float32), np.zeros((1, 2), np.float32)]
    )

    # replicated fat macro-record table (bf16):
    # x[8], y[8], c = oh0-oh2, s = oh1-oh3 per batch
    t2f = np.zeros((vpad, FAT), bf16)
    t2f[:V, 0:8] = positions[:, :, 0].T.astype(bf16)
    t2f[:V, 8:16] = positions[:, :, 1].T.astype(bf16)
    ohT = rotation_onehot.transpose(1, 0, 2)  # (V, B, 4)
    t2f[:V, 16:24] = (ohT[:, :, 0] - ohT[:, :, 2]).astype(bf16)
    t2f[:V, 24:32] = (ohT[:, :, 1] - ohT[:, :, 3]).astype(bf16)

    lengths = (n2p >= 0).sum(axis=1)

    # shard nets contiguously
    per = (N + N_CORES - 1) // N_CORES
    shards = [(c * per, min((c + 1) * per, N)) for c in range(N_CORES)]

    # bucket counts per core -> global G_k
    ks = range(1, M + 1)
    counts = np.zeros((N_CORES, M + 1), np.int64)
    for c, (a, b) in enumerate(shards):
        cnt = np.bincount(lengths[a:b], minlength=M + 1)
        counts[c] = cnt
    gk = {k: int(-(-counts[:, k].max() // P)) for k in ks if counts[:, k].max() > 0}

    # chunk plan: cs = g*k <= 127 so each chunk is one dma_gather
    # (<= 16256 indices; the SWDGE descriptor carveout fits ~16336).
    chunk_plan = []
    slot_off = 0
    g_off = 0
    bucket_offs = {}
    for k in sorted(gk):
        g_total = gk[k]
        gmax = max(1, 127 // k)
        bucket_offs[k] = (slot_off, g_off)
        g_done = 0
        while g_done < g_total:
            g = min(gmax, g_total - g_done)
            chunk_plan.append((k, g, slot_off, g_off))
            slot_off += g * k
            g_off += g
            g_done += g
    tot_slot = slot_off
    tot_g = g_off

    # per-core slot tables
    in_maps = []
    for c, (a, b) in enumerate(shards):
        mac_all = np.full((P, tot_slot), pad_mac, np.int32)
        w_core = np.zeros((P, tot_g), np.float32)
        pin_all = np.full((P, tot_slot), pad_pin, np.int32)
        ln = lengths[a:b]
        for k in sorted(gk):
            so, go = bucket_offs[k]
            sel = np.nonzero(ln == k)[0]
            nk = len(sel)
            if nk == 0:
                continue
            gkk = gk[k]
            ids = n2p[a:b][sel][:, :k]               # (nk, k) valid prefix
            w = net_weights[a:b][sel].astype(np.float32)
            idsp = np.full((gkk * P, k), pad_pin, np.int32)
            idsp[:nk] = ids
            wp_ = np.zeros((gkk * P,), np.float32)
            wp_[:nk] = w
            # net r -> (g=r//P, p=r%P)
            pin_all[:, so:so + gkk * k] = (
                idsp.reshape(gkk, P, k).transpose(1, 0, 2).reshape(P, gkk * k)
            )
            mac_all[:, so:so + gkk * k] = p2m[pin_all[:, so:so + gkk * k]]
            w_core[:, go:go + gkk] = wp_.reshape(gkk, P).T

        # int16 wrapped index stream for dma_gather: list position
        # i = col*128 + p; wrapped tile [128, 8*tot_slot] with
        # tile[q, s] = idx_list[s*16 + q%16].
        idx_list = mac_all.T.ravel().astype(np.int16)         # [tot_slot*128]
        idx16 = np.tile(idx_list.reshape(tot_slot * 8, 16).T, (8, 1))
        # per-slot pin offsets, net-grouped dense stream (bf16)
        t1e = t1pad[pin_all].astype(np.float32).reshape(P, tot_slot * 2)

        in_maps.append({
            "t1e": t1e.astype(bf16), "t2f": t2f,
            "idx16": idx16.astype(np.int16),
            "w_all": w_core.astype(bf16),
        })

    meta = (vpad, ppad, tuple(chunk_plan), tot_slot, tot_g)
    return meta, in_maps


def bfloat16_np():
    import ml_dtypes
    return ml_dtypes.bfloat16


_prog_cache = {}


def kernel(**inputs):
    meta, in_maps = prep_host(
        np.asarray(inputs["positions"]),
        np.asarray(inputs["pin_offsets"]),
        np.asarray(inputs["rotation_onehot"]),
        np.asarray(inputs["net_weights"]),
        np.asarray(inputs["net_to_pin"]),
        np.asarray(inputs["pin_to_macro"]),
    )
    if meta not in _prog_cache:
        _prog_cache[meta] = build_program(*meta)
    nc = _prog_cache[meta]
    res = bass_utils.run_bass_kernel_spmd(nc, in_maps, core_ids=list(range(N_CORES)))
    total = np.zeros(8, np.float64)
    for r in res.results:
        total += r["acc"].astype(np.float64).sum(axis=0)
    return total.astype(np.float32)


# revision 19
# speedup vs baseline: 1.2823x; 1.2329x over previous
"""DifferentiableHPWL on 8 trn2 NeuronCores.

Strategy (sharded by nets, hint-compliant):
  - Host: bucket nets by pin-count, shard nets across 8 cores, compose
    slot->macro = pin_to_macro[net_to_pin] (int16 — macro ids < 32768),
    lay out per-core slot tables, the per-slot pin-offset stream
    (net-grouped layout of the replicated pin_offsets input) and the
    per-macro record table T2[v] = (x[8b], y[8b], c[8b], s[8b]) in bf16
    (64B payload rows on a 256B stride so dma_gather can address them;
    c/s are the one-hot contractions cos/sin of the 90-degree rotations).
  - Device (per core): per chunk of nets, ONE dma_gather instruction
    fetches all ~16K macro records for the chunk (vs. one indirect DMA
    per 128 slots — the SWDGE ~1us/instruction overhead was the original
    3.5ms bottleneck), spread over the 4 SWDGE queues. DVE computes
    rotated pin positions px,py for all 8 batches in bf16 ([net, pin,
    batch] layout, batch innermost so every elementwise op is contiguous),
    per-net max/min (strided-view reduces), exp via ACT (in-place), and
    per-net partial sums into persistent arenas. The cheap per-net tail
    (ln, lse combine, weighting, reduction to [128, 8] f32 partials) runs
    once over the arenas at the end.
  - Host: sum partials over partitions and cores -> (8,) float32.
"""

import numpy as np

import concourse.bass as bass
import concourse.mybir as mybir
from concourse import ap_utils
from concourse.bass_primitives import MemorySpace
from concourse.tile import TileContext
from concourse import bass_utils, library_config

F32 = mybir.dt.float32
BF16 = mybir.dt.bfloat16
I16 = mybir.dt.int16
AX = mybir.AxisListType
ALU = mybir.AluOpType
ACT = mybir.ActivationFunctionType

GAMMA = 10.0
N_CORES = 8
P = 128  # partitions
REC = 32  # record fields (bf16): x[8], y[8], c[8], s[8]
FAT = 128  # fat row stride in bf16 elements (256B, dma_gather requirement)


def _patch_tile_drain():
    """This walrus lowers InstDrain to a TPB_CTRL form with too few sync-wait
    slots; hoist the final drain's waits onto single-wait nops instead."""
    from concourse.vector_clock import ScopedClock

    if getattr(TileContext, "_drain_patched", False):
        return

    def _drain_and_barrier(self, tick_clock, wait_clock):
        nc = self.nc
        carrier = nc.sync.nop(nofuse=True, hint="drain_wait_carrier")
        wait_clock.add_sem_waits(
            carrier.ins, ScopedClock({None: tick_clock.global_clock})
        )
        waits = list(carrier.ins.sync_info.on_wait) if carrier.ins.sync_info else []
        if len(waits) > 1:
            carrier.ins.sync_info = mybir.SyncInfo(on_wait=[waits[0]], on_update=[])
            for w in waits[1:]:
                n2 = nc.sync.nop(nofuse=True, hint="drain_wait_extra")
                n2.ins.sync_info = mybir.SyncInfo(on_wait=[w], on_update=[])
        nc.sync.drain()
        nc.all_engine_barrier()
        popped = nc._tile_sem_poison_stack.pop()
        assert popped is self._sem_poison
        nc.clear_and_free_semaphores(list(self.sems.allocated().values()))
        nc.all_engine_barrier()

    TileContext._drain_and_barrier = _drain_and_barrier
    TileContext._drain_patched = True


def _split_excess_waits(nc, dma_limit=1, other_limit=1):
    """walrus here rejects DMA instructions with >1 sync wait (and drains with
    >1). Hoist excess waits onto same-engine NoOp carriers inserted before the
    instruction — the sequencer executes carrier waits first, preserving
    semantics."""
    ctr = 0
    dma_types = (mybir.InstDMACopy, mybir.InstDrain, mybir.InstDMAGatherAnt)
    for f in nc.m.functions:
        for bb in f.blocks:
            out = []
            changed = False
            for inst in bb.instructions:
                si = inst.sync_info
                waits = list(si.on_wait) if si and si.on_wait else []
                limit = dma_limit if isinstance(inst, dma_types) else other_limit
                if len(waits) > limit:
                    keep = waits[len(waits) - limit:]
                    for w in waits[: len(waits) - limit]:
                        nop = mybir.InstNoOp(name=f"waitsplit-{ctr}")
                        ctr += 1
                        nop.engine = inst.engine
                        nop.sync_info = mybir.SyncInfo(on_wait=[w], on_update=[])
                        nc.register_instruction(nop, overwrite=True)
                        out.append(nop)
                    inst.sync_info = mybir.SyncInfo(
                        on_wait=keep,
                        on_update=list(si.on_update) if si.on_update else [],
                    )
                    changed = True
                out.append(inst)
            if changed:
                bb.instructions = out
    return ctr


def _dma_gather(g, out_ap, in_ap, idxs_ap, num_idxs, elem_size, elem_step,
                queue_num=0, reg_cache=None):
    """nc.gpsimd.dma_gather without the elem%256 assert — the firmware's
    non-transpose path only requires the table row *stride* to be a 256B
    multiple (stride_bytes_256 descriptor field); the transferred elem size
    is free. single_packet=False (concat-all overflows the 64-desc packet
    limit and wedges the SDMA)."""
    nc = g.bass
    assert idxs_ap.dtype == I16
    assert in_ap.space == MemorySpace.DRAM
    assert out_ap.space == MemorySpace.SBUF
    assert idxs_ap.space == MemorySpace.SBUF
    assert ap_utils.ap_is_contiguous(out_ap.ap[1:])
    assert ap_utils.ap_is_contiguous(idxs_ap.ap[1:])
    assert in_ap.ap[-1][1] == elem_size
    assert out_ap.ap[-1][1] == elem_size
    assert out_ap.ap[0][1] * out_ap.ap[1][1] == ((num_idxs + 127) // 128) * 128
    assert in_ap.ap[0][0] == elem_step
    stride_bytes = elem_step * mybir.dt.size(in_ap.dtype)
    assert stride_bytes % 256 == 0 and stride_bytes // 256 < 256
    _in_ap = g.lower_ap_dma(in_ap, for_custom_bir_dma=True)
    _idxs_ap = g.lower_ap(idxs_ap)
    _out_ap = g.lower_ap(out_ap)
    if reg_cache is not None and num_idxs in reg_cache:
        reg = reg_cache[num_idxs]
    else:
        reg = g.to_reg(num_idxs)
        if reg_cache is not None:
            reg_cache[num_idxs] = reg
    return g.add_instruction(
        mybir.InstDMAGatherAnt(
            name=nc.get_next_instruction_name(),
            ins=[*_in_ap, _idxs_ap, g.lower_val_access(reg)],
            outs=[_out_ap],
            transpose=False,
            num_idxs=num_idxs,
            elem_size=elem_size,
            stride_bytes_256=stride_bytes // 256,
            gen_mode=0,
            single_packet=False,
            queue_num=queue_num,
            sbuf_tokens_per_rank=0,
            sbuf_free_dim_per_rank=0,
            sbuf_free_dim_pad_per_rank=0,
            sbuf_byte_offset=0,
        )
    )


def build_program(vpad, ppad, chunk_plan, tot_slot, tot_g, rep=1):
    """Build the SPMD Bass program.

    vpad: padded macro count (multiple of 128); ppad: unused (layout compat).
    chunk_plan: list of (k, g, slot_off, g_off) chunks; cs = g*k <= 127.
    tot_slot: total slots per partition; tot_g: total net-groups/partition.
    rep: repeat the whole compute (timing builds only).
    """
    _patch_tile_drain()
    nc = bass.Bass("TRN2", target_bir_lowering=False, debug=False,
                   num_swdge_queues=4, dynamic_dma_scratch_size=24576)

    t1e = nc.dram_tensor("t1e", [P, tot_slot * 2], BF16, kind="ExternalInput")
    t2f = nc.dram_tensor("t2f", [vpad, FAT], BF16, kind="ExternalInput")
    idx16 = nc.dram_tensor("idx16", [P, tot_slot * 8], I16, kind="ExternalInput")
    w_all = nc.dram_tensor("w_all", [P, tot_g], BF16, kind="ExternalInput")
    out = nc.dram_tensor("acc", [P, 8], F32, kind="ExternalOutput")

    G8 = tot_g * 8

    with TileContext(nc) as tc:
        with (
            tc.tile_pool(name="persist", bufs=1) as pp,
            tc.tile_pool(name="work", bufs=4) as wp,
        ):
            nc.gpsimd.load_library(library_config.mlp)
            # ---- persistent loads ----
            idx_t = pp.tile([P, tot_slot * 8], I16)
            nc.sync.dma_start(idx_t[:], idx16.ap())
            t1_t = pp.tile([P, tot_slot * 2], BF16)
            nc.sync.dma_start(t1_t[:], t1e.ap())
            w_t = pp.tile([P, tot_g], BF16)
            nc.sync.dma_start(w_t[:], w_all.ap())
            acc = pp.tile([P, 8], F32)
            # per-net arenas: [coord][net-group, batch] for Mx/mn/Sx/Sn; the
            # cheap per-net tail math runs once over these per rep instead of
            # as ~10 tiny engine ops per chunk.
            aM = pp.tile([P, 2 * G8], BF16)
            am = pp.tile([P, 2 * G8], BF16)
            aSx = pp.tile([P, 2 * G8], BF16)
            aSn = pp.tile([P, 2 * G8], BF16)

            reg_cache = {}
            # Execute chunks largest-first (short pipeline drain) with
            # greedy LPT queue assignment — chunk sizes vary ~5x, so ci%4
            # round-robin left one SWDGE queue with ~1.7x the descriptors
            # of another.
            order = list(range(len(chunk_plan)))
            qassign = {i: i % 4 for i in order}
            for _ in range(rep):
                # ---- chunk loop ----
                for ci in order:
                    (k, g, slot_off, g_off) = chunk_plan[ci]
                    cs = g * k  # slots per partition this chunk
                    n_idx = cs * P
                    rec = wp.tile([P, cs * REC], BF16, tag="rec")
                    _dma_gather(
                        nc.gpsimd,
                        out_ap=rec.rearrange("p (c e) -> p c e", e=REC),
                        in_ap=t2f.ap()[:, 0:REC],
                        idxs_ap=idx_t[:, 8 * slot_off: 8 * (slot_off + cs)],
                        num_idxs=n_idx, elem_size=REC, elem_step=FAT,
                        queue_num=qassign[ci], reg_cache=reg_cache,
                    )

                    r5 = rec.rearrange("p (g j c) -> p g j c", g=g, j=k)
                    Xv = r5[:, :, :, 0:8]
                    Yv = r5[:, :, :, 8:16]
                    Cv = r5[:, :, :, 16:24]
                    Sv = r5[:, :, :, 24:32]
                    r1f = t1_t[:, 2 * slot_off: 2 * (slot_off + cs)].rearrange(
                        "p (g j f) -> p g j f", g=g, j=k)
                    oxv = r1f[:, :, :, 0:1].to_broadcast([P, g, k, 8])
                    oyv = r1f[:, :, :, 1:2].to_broadcast([P, g, k, 8])

                    # slot tiles are [g, j, b], b innermost: every build op
                    # contiguous; per-net reduces use a [g, b, j] strided view.
                    pv = wp.tile([P, 2 * cs * 8], BF16, tag="pv")
                    ta = wp.tile([P, cs * 8], BF16, tag="ta")
                    tar = ta.rearrange("p (g j b) -> p g j b", g=g, j=k)
                    tb = wp.tile([P, cs * 8], BF16, tag="tb")
                    tbr = tb.rearrange("p (g j b) -> p g j b", g=g, j=k)
                    pvc = [
                        pv[:, c * cs * 8:(c + 1) * cs * 8].rearrange(
                            "p (g j b) -> p g j b", g=g, j=k)
                        for c in range(2)
                    ]

                    nc.vector.tensor_tensor(out=tar, in0=Cv, in1=oxv,
                                            op=ALU.mult)
                    nc.vector.tensor_tensor(out=tbr, in0=Sv, in1=oyv,
                                            op=ALU.mult)
                    nc.vector.tensor_tensor(out=tar, in0=tar, in1=Xv,
                                            op=ALU.add)
                    nc.vector.tensor_tensor(out=pvc[0], in0=tar, in1=tbr,
                                            op=ALU.subtract)
                    nc.vector.tensor_tensor(out=tar, in0=Sv, in1=oxv,
                                            op=ALU.mult)
                    nc.vector.tensor_tensor(out=tbr, in0=Cv, in1=oyv,
                                            op=ALU.mult)
                    nc.vector.tensor_tensor(out=tar, in0=tar, in1=Yv,
                                            op=ALU.add)
                    nc.vector.tensor_tensor(out=pvc[1], in0=tar, in1=tbr,
                                            op=ALU.add)

                    for c in range(2):
                        off = c * G8 + g_off * 8
                        Mx = aM[:, off:off + g * 8]
                        mn = am[:, off:off + g * 8]
                        src_t = pvc[c].transpose([0, 1, 3, 2])  # [P, g, b, j]
                        nc.vector.tensor_reduce(out=Mx, in_=src_t, axis=AX.X,
                                                op=ALU.max)
                        nc.vector.tensor_reduce(out=mn, in_=src_t, axis=AX.X,
                                                op=ALU.min)
                        Mb = Mx.rearrange("p (g b) -> p g b", g=g).unsqueeze(
                            2).to_broadcast([P, g, k, 8])
                        mb = mn.rearrange("p (g b) -> p g b", g=g).unsqueeze(
                            2).to_broadcast([P, g, k, 8])
                        d1 = wp.tile([P, cs * 8], BF16, tag=f"d1{c}")
                        d1r = d1.rearrange("p (g j b) -> p g j b", g=g, j=k)
                        d1t = d1r.transpose([0, 1, 3, 2])
                        d2 = wp.tile([P, cs * 8], BF16, tag=f"d2{c}")
                        d2r = d2.rearrange("p (g j b) -> p g j b", g=g, j=k)
                        d2t = d2r.transpose([0, 1, 3, 2])
                        nc.vector.tensor_tensor(out=d1r, in0=pvc[c], in1=Mb,
                                                op=ALU.subtract)
                        nc.scalar.activation(out=d1[:], in_=d1[:],
                                             func=ACT.Exp, scale=GAMMA)
                        nc.vector.tensor_tensor(out=d2r, in0=pvc[c], in1=mb,
                                                op=ALU.subtract)
                        nc.scalar.activation(out=d2[:], in_=d2[:],
                                             func=ACT.Exp, scale=-GAMMA)
                        with nc.allow_low_precision(reason="<=16-term sums"):
                            nc.vector.tensor_reduce(
                                out=aSx[:, off:off + g * 8], in_=d1t,
                                axis=AX.X, op=ALU.add)
                            nc.vector.tensor_reduce(
                                out=aSn[:, off:off + g * 8], in_=d2t,
                                axis=AX.X, op=ALU.add)

                # ---- per-net tail, once over the whole arenas ----
                nc.scalar.activation(out=aSx[:], in_=aSx[:], func=ACT.Ln)
                nc.scalar.activation(out=aSn[:], in_=aSn[:], func=ACT.Ln)
                nc.vector.tensor_tensor(out=aSx[:], in0=aSx[:], in1=aSn[:],
                                        op=ALU.add)
                nc.vector.tensor_tensor(out=aM[:], in0=aM[:], in1=am[:],
                                        op=ALU.subtract)
                nc.vector.scalar_tensor_tensor(
                    out=aM[:], in0=aSx[:], scalar=1.0 / GAMMA, in1=aM[:],
                    op0=ALU.mult, op1=ALU.add,
                )
                nc.vector.tensor_tensor(out=aM[:, 0:G8], in0=aM[:, 0:G8],
                                        in1=aM[:, G8:2 * G8], op=ALU.add)
                wbig = w_t.unsqueeze(2).to_broadcast([P, tot_g, 8])
                wl = aM[:, 0:G8].rearrange("p (g b) -> p g b", g=tot_g)
                nc.vector.tensor_tensor(out=wl, in0=wl, in1=wbig, op=ALU.mult)
                nc.vector.tensor_reduce(
                    out=acc[:], in_=wl.transpose([0, 2, 1]), axis=AX.X,
                    op=ALU.add,
                )

            nc.sync.dma_start(out.ap(), acc[:])
    _split_excess_waits(nc)
    from concourse.library_overlay import lower_extended_insts
    lower_extended_insts(nc)
    return nc


def prep_host(positions, pin_offsets, rotation_onehot, net_weights,
              net_to_pin, pin_to_macro):
    """Host-side sharding/layout. Returns (meta, in_maps)."""
    B, V, _ = positions.shape
    Pn = pin_offsets.shape[0]
    N, M = net_to_pin.shape
    bf16 = bfloat16_np()

    vpad = ((V + 1 + P - 1) // P) * P  # +1 pad macro row
    ppad = Pn + 1                      # +1 pad pin row
    pad_mac = V
    pad_pin = Pn

    n2p = net_to_pin.astype(np.int32)
    p2m = np.concatenate(
        [pin_to_macro.astype(np.int32), np.array([pad_mac], np.int32)]
    )
    t1pad = np.concatenate(
        [pin_offsets.astype(np.float32), np.zeros((1, 2), np.float32)]
    )

    # replicated fat macro-record table (bf16):
    # x[8], y[8], c = oh0-oh2, s = oh1-oh3 per batch
    t2f = np.zeros((vpad, FAT), bf16)
    t2f[:V, 0:8] = positions[:, :, 0].T.astype(bf16)
    t2f[:V, 8:16] = positions[:, :, 1].T.astype(bf16)
    ohT = rotation_onehot.transpose(1, 0, 2)  # (V, B, 4)
    t2f[:V, 16:24] = (ohT[:, :, 0] - ohT[:, :, 2]).astype(bf16)
    t2f[:V, 24:32] = (ohT[:, :, 1] - ohT[:, :, 3]).astype(bf16)

    lengths = (n2p >= 0).sum(axis=1)

    # shard nets contiguously
    per = (N + N_CORES - 1) // N_CORES
    shards = [(c * per, min((c + 1) * per, N)) for c in range(N_CORES)]

    # bucket counts per core -> global G_k
    ks = range(1, M + 1)
    counts = np.zeros((N_CORES, M + 1), np.int64)
    for c, (a, b) in enumerate(shards):
        cnt = np.bincount(lengths[a:b], minlength=M + 1)
        counts[c] = cnt
    gk = {k: int(-(-counts[:, k].max() // P)) for k in ks if counts[:, k].max() > 0}

    # chunk plan: cs = g*k <= 127 so each chunk is one dma_gather
    # (<= 16256 indices; the SWDGE descriptor carveout fits ~16336).
    chunk_plan = []
    slot_off = 0
    g_off = 0
    bucket_offs = {}
    for k in sorted(gk):
        g_total = gk[k]
        gmax = max(1, 127 // k)
        bucket_offs[k] = (slot_off, g_off)
        g_done = 0
        while g_done < g_total:
            g = min(gmax, g_total - g_done)
            chunk_plan.append((k, g, slot_off, g_off))
            slot_off += g * k
            g_off += g
            g_done += g
    tot_slot = slot_off
    tot_g = g_off

    # per-core slot tables
    in_maps = []
    for c, (a, b) in enumerate(shards):
        mac_all = np.full((P, tot_slot), pad_mac, np.int32)
        w_core = np.zeros((P, tot_g), np.float32)
        pin_all = np.full((P, tot_slot), pad_pin, np.int32)
        ln = lengths[a:b]
        for k in sorted(gk):
            so, go = bucket_offs[k]
            sel = np.nonzero(ln == k)[0]
            nk = len(sel)
            if nk == 0:
                continue
            gkk = gk[k]
            ids = n2p[a:b][sel][:, :k]               # (nk, k) valid prefix
            w = net_weights[a:b][sel].astype(np.float32)
            idsp = np.full((gkk * P, k), pad_pin, np.int32)
            idsp[:nk] = ids
            wp_ = np.zeros((gkk * P,), np.float32)
            wp_[:nk] = w
            # net r -> (g=r//P, p=r%P)
            pin_all[:, so:so + gkk * k] = (
                idsp.reshape(gkk, P, k).transpose(1, 0, 2).reshape(P, gkk * k)
            )
            mac_all[:, so:so + gkk * k] = p2m[pin_all[:, so:so + gkk * k]]
            w_core[:, go:go + gkk] = wp_.reshape(gkk, P).T

        # int16 wrapped index stream for dma_gather: list position
        # i = col*128 + p; wrapped tile [128, 8*tot_slot] with
        # tile[q, s] = idx_list[s*16 + q%16].
        idx_list = mac_all.T.ravel().astype(np.int16)         # [tot_slot*128]
        idx16 = np.tile(idx_list.reshape(tot_slot * 8, 16).T, (8, 1))
        # per-slot pin offsets, net-grouped dense stream (bf16)
        t1e = t1pad[pin_all].astype(np.float32).reshape(P, tot_slot * 2)

        in_maps.append({
            "t1e": t1e.astype(bf16), "t2f": t2f,
            "idx16": idx16.astype(np.int16),
            "w_all": w_core.astype(bf16),
        })

    meta = (vpad, ppad, tuple(chunk_plan), tot_slot, tot_g)
    return meta, in_maps


def bfloat16_np():
    import ml_dtypes
    return ml_dtypes.bfloat16


_prog_cache = {}


def kernel(**inputs):
    meta, in_maps = prep_host(
        np.asarray(inputs["positions"]),
        np.asarray(inputs["pin_offsets"]),
        np.asarray(inputs["rotation_onehot"]),
        np.asarray(inputs["net_weights"]),
        np.asarray(inputs["net_to_pin"]),
        np.asarray(inputs["pin_to_macro"]),
    )
    if meta not in _prog_cache:
        _prog_cache[meta] = build_program(*meta)
    nc = _prog_cache[meta]
    res = bass_utils.run_bass_kernel_spmd(nc, in_maps, core_ids=list(range(N_CORES)))
    total = np.zeros(8, np.float64)
    for r in res.results:
        total += r["acc"].astype(np.float64).sum(axis=0)
    return total.astype(np.float32)
